# revision 12
# baseline (speedup 1.0000x reference)
"""BERT self-attention (B=4, S=2048, HID=768, 12 heads) on 8 NeuronCores.

Sharding: data-parallel over batch (4) x tensor-parallel over heads (2 groups
of 6 heads) -> 8 cores, no cross-core communication.

Design (v2):
- Everything the PE touches is bf16 (1 cyc/col, no fp32r >=256-col constraint,
  half the LDWEIGHTS stream); accumulation stays fp32 in PSUM.  Host pre-casts
  hs/W to bf16 and pre-TRANSPOSES hs, removing all on-device hs transposes.
- exp is split between the Activation engine (native Exp, mask as bias) and
  the DVE (Schraudolph: bf16(exp(s)) bits ~= uint16(s*128/ln2 + 16250.5
  + 128/ln2*mask), one fused tensor_scalar mult+add with uint16 output
  bitcast onto the bf16 probs tile).  ~3% max relative error on the DVE
  share, which the self-normalizing softmax denominator mostly cancels.
- K^T is written by the projection copies directly into per-head zero-padded
  stationaries (full [128,128] tile configs keep the PE's background weight
  loader active; partial configs serialize ~3x).
- ctx^T accumulated with stationary = a 128-wide window of the flat
  [v_h|1|v_h+1|1|...] V layout whose ones column yields the softmax
  denominator as output row 64; PE-transposed back to seq-major and
  normalized by the DVE reciprocal.
- PSUM->SBUF copies ride on the otherwise-idle GpSimd engine.
- Emission order software-pipelines the projections into the attention
  stream: qk pair 0 -> scores(0,0) -> V projection (while exp(0,0) runs) ->
  ctx(0,0) -> scores(0,1) -> qk pair 1 -> tails -> ctx(0,1) -> ... so no
  engine ever waits long.
"""

import numpy as np

import concourse.bacc as bacc
import concourse.mybir as mybir
import concourse.tile as tile
from concourse.bass_utils import run_bass_kernel_spmd
from concourse.masks import make_identity

F32 = mybir.dt.float32
BF16 = mybir.dt.bfloat16
U16 = mybir.dt.uint16
EXP = mybir.ActivationFunctionType.Exp
MULT = mybir.AluOpType.mult
ADD = mybir.AluOpType.add

B = 4
S = 2048
HID = 768
NH_FULL = 12
HD = 64
NCORES = 8
NH = 6              # heads per core
D3 = NH * HD        # 384, per-core projection width
ST = S // 128       # 16 seq tiles
QB = 1024           # query block (2 x 512 matmul chunks)
QC = 512            # max moving-operand width per matmul
NQB = S // QB       # 2
KC = S // 128       # 16 key chunks
VW = NH * (HD + 1) + 63  # 453: flat [v|1]x6 + zero tail

# Schraudolph exp in bf16-bits domain: bf16_bits(exp(s)) ~ s*A + B0
SCHR_A = float(128.0 / np.log(2.0))       # 184.6650...
SCHR_B = float(127 * 128 - 7.366)         # offset calibrated for ZERO-MEAN rel err
# (zero mean so mixed exact/approx chunks don't skew the shared denominator)
# kc chunks handled by the Activation engine (rest go to DVE Schraudolph)
ACT_KCS = frozenset((0, 2, 4, 6, 8, 10, 12, 14, 15))

_nc_cache: dict = {}


def _build(ck: int):
    """Build the per-core program. ck = # of 128-row contraction chunks in the
    projection (6 plain, 7 when biases are folded in via an augmented row)."""
    nc = bacc.Bacc("TRN2", target_bir_lowering=False, debug=False)
    hst_d = nc.dram_tensor("hst", [ck * 128, S], BF16, kind="ExternalInput")
    wq_d = nc.dram_tensor("wq", [ck * 128, D3], BF16, kind="ExternalInput")
    wk_d = nc.dram_tensor("wk", [ck * 128, D3], BF16, kind="ExternalInput")
    wv_d = nc.dram_tensor("wv", [ck * 128, D3], BF16, kind="ExternalInput")
    mask_d = nc.dram_tensor("mask", [128, KC], F32, kind="ExternalInput")
    msch_d = nc.dram_tensor("msch", [128, KC], F32, kind="ExternalInput")
    out_d = nc.dram_tensor("out", [S, D3], F32, kind="ExternalOutput")

    with tile.TileContext(nc) as tc:
        with (
            tc.tile_pool(name="const", bufs=1) as constp,
            tc.tile_pool(name="qkpool", bufs=1) as qkp,
            tc.tile_pool(name="vpool", bufs=1) as vp,
            tc.tile_pool(name="hstpool", bufs=1) as hstp,
            tc.tile_pool(name="wpool", bufs=1) as wp,
            tc.tile_pool(name="outpool", bufs=1) as outp,
            tc.tile_pool(name="prpool", bufs=2) as prp,
            tc.tile_pool(name="ctxtpool", bufs=3) as ctxtp,
            tc.tile_pool(name="rdpool", bufs=4) as rdp,
            tc.tile_pool(name="stps", bufs=3, space="PSUM") as stps,
            tc.tile_pool(name="wps", bufs=2, space="PSUM") as wps,
        ):
            identity = constp.tile([128, 128], BF16)
            make_identity(nc, identity)
            mask_sb = constp.tile([128, KC], F32)
            msch_sb = constp.tile([128, KC], F32)

            hsT = [hstp.tile([128, S], BF16, name=f"hsT{c}") for c in range(ck)]
            wq_sb = wp.tile([128, ck, D3], BF16, name="wq_sb")
            wk_sb = wp.tile([128, ck, D3], BF16, name="wk_sb")
            wv_sb = wp.tile([128, ck, D3], BF16, name="wv_sb")

            qt = [qkp.tile([128, S], BF16, name=f"qt{m}") for m in range(3)]
            # per-head zero-padded K^T stationaries (head h: real rows in the
            # h%2 half, zeros in the other so the other head's q-rows in the
            # shared moving operand contribute nothing)
            ktp = [qkp.tile([128, S], BF16, name=f"ktp{h}") for h in range(NH)]
            for h in range(NH):
                half = ktp[h][64:128, :] if h % 2 == 0 else ktp[h][0:64, :]
                nc.vector.memset(half.bitcast(U16), 0)

            v_sb = [vp.tile([128, VW], BF16, name=f"v{i}") for i in range(ST)]
            for i in range(ST):
                v3 = v_sb[i][:, 0:NH * (HD + 1)].rearrange("p (h e) -> p h e", h=NH)
                nc.gpsimd.memset(v3[:, :, HD:HD + 1], 1.0)
                nc.gpsimd.memset(v_sb[i][:, NH * (HD + 1):VW].bitcast(U16), 0)

            ob = [outp.tile([128, ST // 2, D3], F32, name=f"ob{i}") for i in range(2)]
            out_sb = [ob[i // (ST // 2)][:, i % (ST // 2), :] for i in range(ST)]

            # ---- input DMAs (SP queue), ordered by first use; hsT arrives
            # in 512-col pieces so the pair-0 projection starts ~7us in ----
            nc.sync.dma_start(wq_sb[:], wq_d.ap().rearrange("(c p) n -> p c n", p=128))
            nc.sync.dma_start(mask_sb[:], mask_d[:])
            nc.sync.dma_start(msch_sb[:], msch_d[:])
            hst_r = hst_d.ap().rearrange("(c p) s -> p c s", p=128)
            for n in range(S // QC):
                for c in range(ck):
                    sl = slice(n * QC, (n + 1) * QC)
                    nc.sync.dma_start(hsT[c][:, sl], hst_r[:, c, sl])
                if n == 0:
                    nc.sync.dma_start(
                        wk_sb[:], wk_d.ap().rearrange("(c p) n -> p c n", p=128))
            nc.sync.dma_start(wv_sb[:], wv_d.ap().rearrange("(c p) n -> p c n", p=128))

            # ---- emission helpers ----
            def emit_qk_pair(m):
                """Project q/k for head pair m into qt[m] / padded ktp[2m,2m+1]."""
                for n in range(S // QC):
                    for which, w_sb in (("q", wq_sb), ("k", wk_sb)):
                        ps = wps.tile([128, QC], F32, name="ps", tag="mm")
                        for c in range(ck):
                            nc.tensor.matmul(
                                ps[:],
                                w_sb[:, c, m * 128:(m + 1) * 128],
                                hsT[c][:, n * QC:(n + 1) * QC],
                                start=(c == 0),
                                stop=(c == ck - 1),
                            )
                        sl = slice(n * QC, (n + 1) * QC)
                        if which == "q":
                            nc.scalar.copy(qt[m][:, sl], ps[:])
                        else:
                            nc.scalar.copy(ktp[2 * m][0:64, sl], ps[0:64, :])
                            nc.scalar.copy(ktp[2 * m + 1][64:128, sl], ps[64:128, :])

            def emit_v_tile(st):
                vps = wps.tile([128, QC], F32, name="vps", tag="mm")
                for c in range(ck):
                    nc.tensor.matmul(
                        vps[:, 0:D3],
                        hsT[c][:, st * 128:(st + 1) * 128],
                        wv_sb[:, c, :],
                        start=(c == 0),
                        stop=(c == ck - 1),
                    )
                v3 = v_sb[st][:, 0:NH * (HD + 1)].rearrange("p (h e) -> p h e", h=NH)
                nc.scalar.copy(
                    v3[:, :, 0:HD], vps[:, 0:D3].rearrange("p (h d) -> p h d", h=NH)
                )

            def emit_scores(hp, qb, hh):
                """scores^T + exp for head 2*hp+hh, query block qb -> pr tile."""
                pr = prp.tile([128, KC, QB], BF16, name="pr")
                h = 2 * hp + hh
                for kc in range(KC):
                    sps = stps.tile([128, QB], F32, name="sps", tag="sc")
                    for qc in range(QB // QC):
                        nc.tensor.matmul(
                            sps[:, qc * QC:(qc + 1) * QC],
                            ktp[h][:, kc * 128:(kc + 1) * 128],
                            qt[hp][:, qb * QB + qc * QC:qb * QB + (qc + 1) * QC],
                        )
                    if kc in ACT_KCS:
                        nc.scalar.activation(
                            pr[:, kc, :], sps[:], EXP,
                            bias=mask_sb[:, kc:kc + 1], scale=1.0,
                        )
                    else:
                        nc.vector.tensor_scalar(
                            pr[:, kc, :].bitcast(U16), sps[:],
                            SCHR_A, msch_sb[:, kc:kc + 1],
                            op0=MULT, op1=ADD,
                        )
                return pr

            def emit_ctx(hp, qb, hh, pr):
                """ctx^T (+denominator row 64) for head 2*hp+hh, block qb."""
                h = 2 * hp + hh
                ctxt = ctxtp.tile([HD + 1, QB], BF16, name="ctxt")
                for qc in range(QB // QC):
                    cps = stps.tile([128, QC], F32, name="cps", tag="sc")
                    for kc in range(KC):
                        nc.tensor.matmul(
                            cps[:],
                            v_sb[kc][:, h * (HD + 1):h * (HD + 1) + 128],
                            pr[:, kc, qc * QC:(qc + 1) * QC],
                            start=(kc == 0),
                            stop=(kc == KC - 1),
                        )
                    nc.vector.tensor_copy(
                        ctxt[:, qc * QC:(qc + 1) * QC], cps[0:HD + 1, :]
                    )
                return ctxt

            def emit_tail(h, qb, ctxt):
                """transpose ctx^T back to seq-major, normalize into out_sb."""
                E = HD + 2   # 66: keeps each slice 4-byte aligned in PSUM
                tp2 = wps.tile([128, (QB // 128) * E], BF16, name="tp2", tag="mm")
                for qs in range(QB // 128):
                    nc.tensor.transpose(
                        tp2[:, qs * E:qs * E + HD + 1],
                        ctxt[:, qs * 128:(qs + 1) * 128],
                        identity[0:HD + 1, 0:HD + 1],
                    )
                rd = rdp.tile([128, QB // 128], F32, name="rd")
                nc.vector.reciprocal(rd[:], tp2[:, HD::E])
                for qs in range(QB // 128):
                    sti = qb * (QB // 128) + qs
                    nc.vector.tensor_scalar_mul(
                        out_sb[sti][:, h * HD:(h + 1) * HD],
                        tp2[:, qs * E:qs * E + HD],
                        rd[:, qs:qs + 1],
                    )

            out_r = out_d.ap().rearrange("(t p) n -> p t n", p=128)
            def emit_out_dma(qb):
                half = ST // 2
                nc.sync.dma_start(out_r[:, qb * half:(qb + 1) * half, :], ob[qb][:])

            # ---- main schedule ----
            emit_qk_pair(0)
            pending = []        # deferred (h, qb, ctxt) tails
            fillers = [lambda m=m: emit_qk_pair(m) for m in (1, 2)] + [None] * 4

            for hp in range(NH // 2):
                for qb in range(NQB):
                    prs = [emit_scores(hp, qb, hh) for hh in range(2)]
                    if hp == 0 and qb == 0:
                        for st in range(ST):
                            emit_v_tile(st)
                    else:
                        f = fillers.pop(0)
                        if f is not None:
                            f()
                    for args in pending:
                        emit_tail(*args)
                    if pending and hp == NH // 2 - 1 and qb == NQB - 1:
                        emit_out_dma(0)   # qb0 outputs now complete
                    pending = []
                    for hh in range(2):
                        ctxt = emit_ctx(hp, qb, hh, prs[hh])
                        if hp == NH // 2 - 1 and qb == NQB - 1:
                            emit_tail(2 * hp + hh, qb, ctxt)
                        else:
                            pending.append((2 * hp + hh, qb, ctxt))
            emit_out_dma(1)

    nc.compile()
    return nc


def _get_nc(ck: int):
    if ck not in _nc_cache:
        _nc_cache[ck] = _build(ck)
    return _nc_cache[ck]


def _prepare_in_maps(hidden_states, attention_mask, Wq, bq, Wk, bk, Wv, bv):
    bf16 = mybir.dt.np(BF16)
    hs = np.asarray(hidden_states, dtype=np.float32)
    mask = np.asarray(attention_mask, dtype=np.float32).reshape(B, S)
    wq = np.asarray(Wq, dtype=np.float32) * np.float32(0.125)  # fold 1/sqrt(HD)
    wk = np.asarray(Wk, dtype=np.float32)
    wv = np.asarray(Wv, dtype=np.float32)
    bqs = np.asarray(bq, dtype=np.float32) * np.float32(0.125)
    bks = np.asarray(bk, dtype=np.float32)
    bvs = np.asarray(bv, dtype=np.float32)

    if bqs.any() or bks.any() or bvs.any():
        ck = 7
        pad = ck * 128 - (HID + 1)
        ones = np.ones((B, S, 1), np.float32)
        zer = np.zeros((B, S, pad), np.float32)
        hs = np.concatenate([hs, ones, zer], axis=2)
        def aug(w, b):
            return np.concatenate(
                [w, b[None, :], np.zeros((pad, HID), np.float32)], axis=0)
        wq, wk, wv = aug(wq, bqs), aug(wk, bks), aug(wv, bvs)
    else:
        ck = 6

    wq16 = wq.astype(bf16)
    wk16 = wk.astype(bf16)
    wv16 = wv.astype(bf16)
    msch = (np.float32(SCHR_B) + np.float32(SCHR_A) * mask).astype(np.float32)

    in_maps = []
    for core in range(NCORES):
        b, hg = core // 2, core % 2
        cols = slice(hg * D3, (hg + 1) * D3)
        in_maps.append({
            "hst": np.ascontiguousarray(hs[b].T.astype(bf16)),
            "wq": np.ascontiguousarray(wq16[:, cols]),
            "wk": np.ascontiguousarray(wk16[:, cols]),
            "wv": np.ascontiguousarray(wv16[:, cols]),
            "mask": np.ascontiguousarray(mask[b].reshape(KC, 128).T),
            "msch": np.ascontiguousarray(msch[b].reshape(KC, 128).T),
        })
    return ck, in_maps


def run(hidden_states, attention_mask, Wq, bq, Wk, bk, Wv, bv, **rb_kwargs):
    """Shard, run on 8 cores, gather. Returns (output, BassKernelResults)."""
    ck, in_maps = _prepare_in_maps(
        hidden_states, attention_mask, Wq, bq, Wk, bk, Wv, bv
    )
    nc = _get_nc(ck)
    res = run_bass_kernel_spmd(nc, in_maps, core_ids=list(range(NCORES)), **rb_kwargs)
    out = np.empty((B, S, HID), dtype=np.float32)
    for core in range(NCORES):
        b, hg = core // 2, core % 2
        out[b, :, hg * D3:(hg + 1) * D3] = res.results[core]["out"]
    return out, res


def kernel(hidden_states, attention_mask, Wq, bq, Wk, bk, Wv, bv):
    out, _ = run(hidden_states, attention_mask, Wq, bq, Wk, bk, Wv, bv)
    return out


# revision 13
# speedup vs baseline: 1.0300x; 1.0300x over previous
"""BERT self-attention (B=4, S=2048, HID=768, 12 heads) on 8 NeuronCores.

Sharding: data-parallel over batch (4) x tensor-parallel over heads (2 groups
of 6 heads) -> 8 cores, no cross-core communication.

Design (v2):
- Everything the PE touches is bf16 (1 cyc/col, no fp32r >=256-col constraint,
  half the LDWEIGHTS stream); accumulation stays fp32 in PSUM.  Host pre-casts
  hs/W to bf16 and pre-TRANSPOSES hs, removing all on-device hs transposes.
- exp is split between the Activation engine (native Exp, mask as bias) and
  the DVE (Schraudolph: bf16(exp(s)) bits ~= uint16(s*128/ln2 + 16250.5
  + 128/ln2*mask), one fused tensor_scalar mult+add with uint16 output
  bitcast onto the bf16 probs tile).  ~3% max relative error on the DVE
  share, which the self-normalizing softmax denominator mostly cancels.
- K^T is written by the projection copies directly into per-head zero-padded
  stationaries (full [128,128] tile configs keep the PE's background weight
  loader active; partial configs serialize ~3x).
- ctx^T accumulated with stationary = a 128-wide window of the flat
  [v_h|1|v_h+1|1|...] V layout whose ones column yields the softmax
  denominator as output row 64; PE-transposed back to seq-major and
  normalized by the DVE reciprocal.
- PSUM->SBUF copies ride on the otherwise-idle GpSimd engine.
- Emission order software-pipelines the projections into the attention
  stream: qk pair 0 -> scores(0,0) -> V projection (while exp(0,0) runs) ->
  ctx(0,0) -> scores(0,1) -> qk pair 1 -> tails -> ctx(0,1) -> ... so no
  engine ever waits long.
"""

import numpy as np

import concourse.bacc as bacc
import concourse.mybir as mybir
import concourse.tile as tile
from concourse.bass_utils import run_bass_kernel_spmd
from concourse.masks import make_identity

F32 = mybir.dt.float32
BF16 = mybir.dt.bfloat16
U16 = mybir.dt.uint16
EXP = mybir.ActivationFunctionType.Exp
MULT = mybir.AluOpType.mult
ADD = mybir.AluOpType.add

B = 4
S = 2048
HID = 768
NH_FULL = 12
HD = 64
NCORES = 8
NH = 6              # heads per core
D3 = NH * HD        # 384, per-core projection width
ST = S // 128       # 16 seq tiles
QB = 1024           # query block (2 x 512 matmul chunks)
QC = 512            # max moving-operand width per matmul
NQB = S // QB       # 2
KC = S // 128       # 16 key chunks
VW = NH * (HD + 1) + 63  # 453: flat [v|1]x6 + zero tail

# Schraudolph exp in bf16-bits domain: bf16_bits(exp(s)) ~ s*A + B0
SCHR_A = float(128.0 / np.log(2.0))       # 184.6650...
SCHR_B = float(127 * 128 - 7.366)         # offset calibrated for ZERO-MEAN rel err
# (zero mean so mixed exact/approx chunks don't skew the shared denominator)
# kc chunks handled by the Activation engine (rest go to DVE Schraudolph)
ACT_KCS = frozenset((0, 2, 4, 6, 8, 10, 12, 14, 15))

_nc_cache: dict = {}


def _build(ck: int):
    """Build the per-core program. ck = # of 128-row contraction chunks in the
    projection (6 plain, 7 when biases are folded in via an augmented row)."""
    nc = bacc.Bacc("TRN2", target_bir_lowering=False, debug=False)
    hst_d = nc.dram_tensor("hst", [ck * 128, S], BF16, kind="ExternalInput")
    wq_d = nc.dram_tensor("wq", [ck * 128, D3], BF16, kind="ExternalInput")
    wk_d = nc.dram_tensor("wk", [ck * 128, D3], BF16, kind="ExternalInput")
    wv_d = nc.dram_tensor("wv", [ck * 128, D3], BF16, kind="ExternalInput")
    mask_d = nc.dram_tensor("mask", [128, KC], F32, kind="ExternalInput")
    msch_d = nc.dram_tensor("msch", [128, KC], F32, kind="ExternalInput")
    out_d = nc.dram_tensor("out", [S, D3], F32, kind="ExternalOutput")

    with tile.TileContext(nc) as tc:
        with (
            tc.tile_pool(name="const", bufs=1) as constp,
            tc.tile_pool(name="qkpool", bufs=1) as qkp,
            tc.tile_pool(name="vpool", bufs=1) as vp,
            tc.tile_pool(name="hstpool", bufs=1) as hstp,
            tc.tile_pool(name="wpool", bufs=1) as wp,
            tc.tile_pool(name="outpool", bufs=1) as outp,
            tc.tile_pool(name="prpool", bufs=2) as prp,
            tc.tile_pool(name="ctxtpool", bufs=3) as ctxtp,
            tc.tile_pool(name="rdpool", bufs=4) as rdp,
            tc.tile_pool(name="stps", bufs=2, space="PSUM") as stps,
            tc.tile_pool(name="ctps", bufs=2, space="PSUM") as ctps,
            tc.tile_pool(name="wps", bufs=2, space="PSUM") as wps,
        ):
            identity = constp.tile([128, 128], BF16)
            make_identity(nc, identity)
            mask_sb = constp.tile([128, KC], F32)
            msch_sb = constp.tile([128, KC], F32)

            hsT = [hstp.tile([128, S], BF16, name=f"hsT{c}") for c in range(ck)]
            wq_sb = wp.tile([128, ck, D3], BF16, name="wq_sb")
            wk_sb = wp.tile([128, ck, D3], BF16, name="wk_sb")
            wv_sb = wp.tile([128, ck, D3], BF16, name="wv_sb")

            qt = [qkp.tile([128, S], BF16, name=f"qt{m}") for m in range(3)]
            # per-head zero-padded K^T stationaries (head h: real rows in the
            # h%2 half, zeros in the other so the other head's q-rows in the
            # shared moving operand contribute nothing)
            ktp = [qkp.tile([128, S], BF16, name=f"ktp{h}") for h in range(NH)]
            for h in range(NH):
                half = ktp[h][64:128, :] if h % 2 == 0 else ktp[h][0:64, :]
                nc.vector.memset(half.bitcast(U16), 0)

            v_sb = [vp.tile([128, VW], BF16, name=f"v{i}") for i in range(ST)]
            for i in range(ST):
                v3 = v_sb[i][:, 0:NH * (HD + 1)].rearrange("p (h e) -> p h e", h=NH)
                nc.gpsimd.memset(v3[:, :, HD:HD + 1], 1.0)
                nc.gpsimd.memset(v_sb[i][:, NH * (HD + 1):VW].bitcast(U16), 0)

            ob = [outp.tile([128, ST // 2, D3], F32, name=f"ob{i}") for i in range(2)]
            out_sb = [ob[i // (ST // 2)][:, i % (ST // 2), :] for i in range(ST)]

            # ---- input DMAs (SP queue), ordered by first use; hsT arrives
            # in 512-col pieces so the pair-0 projection starts ~7us in ----
            nc.sync.dma_start(wq_sb[:], wq_d.ap().rearrange("(c p) n -> p c n", p=128))
            nc.sync.dma_start(mask_sb[:], mask_d[:])
            nc.sync.dma_start(msch_sb[:], msch_d[:])
            hst_r = hst_d.ap().rearrange("(c p) s -> p c s", p=128)
            for n in range(S // QC):
                for c in range(ck):
                    sl = slice(n * QC, (n + 1) * QC)
                    nc.sync.dma_start(hsT[c][:, sl], hst_r[:, c, sl])
                if n == 0:
                    nc.sync.dma_start(
                        wk_sb[:], wk_d.ap().rearrange("(c p) n -> p c n", p=128))
            nc.sync.dma_start(wv_sb[:], wv_d.ap().rearrange("(c p) n -> p c n", p=128))

            # ---- emission helpers ----
            def emit_qk_pair(m):
                """Project q/k for head pair m into qt[m] / padded ktp[2m,2m+1]."""
                for n in range(S // QC):
                    for which, w_sb in (("q", wq_sb), ("k", wk_sb)):
                        ps = wps.tile([128, QC], F32, name="ps", tag="mm")
                        for c in range(ck):
                            nc.tensor.matmul(
                                ps[:],
                                w_sb[:, c, m * 128:(m + 1) * 128],
                                hsT[c][:, n * QC:(n + 1) * QC],
                                start=(c == 0),
                                stop=(c == ck - 1),
                            )
                        sl = slice(n * QC, (n + 1) * QC)
                        if which == "q":
                            nc.scalar.copy(qt[m][:, sl], ps[:])
                        else:
                            nc.scalar.copy(ktp[2 * m][0:64, sl], ps[0:64, :])
                            nc.scalar.copy(ktp[2 * m + 1][64:128, sl], ps[64:128, :])

            def emit_v_tile(st):
                vps = wps.tile([128, QC], F32, name="vps", tag="mm")
                for c in range(ck):
                    nc.tensor.matmul(
                        vps[:, 0:D3],
                        hsT[c][:, st * 128:(st + 1) * 128],
                        wv_sb[:, c, :],
                        start=(c == 0),
                        stop=(c == ck - 1),
                    )
                v3 = v_sb[st][:, 0:NH * (HD + 1)].rearrange("p (h e) -> p h e", h=NH)
                nc.scalar.copy(
                    v3[:, :, 0:HD], vps[:, 0:D3].rearrange("p (h d) -> p h d", h=NH)
                )

            def emit_scores(hp, qb, hh):
                """scores^T + exp for head 2*hp+hh, query block qb -> pr tile."""
                pr = prp.tile([128, KC, QB], BF16, name="pr")
                h = 2 * hp + hh
                for kc in range(KC):
                    sps = stps.tile([128, QB], F32, name="sps", tag="sc")
                    for qc in range(QB // QC):
                        nc.tensor.matmul(
                            sps[:, qc * QC:(qc + 1) * QC],
                            ktp[h][:, kc * 128:(kc + 1) * 128],
                            qt[hp][:, qb * QB + qc * QC:qb * QB + (qc + 1) * QC],
                        )
                    if kc in ACT_KCS:
                        nc.scalar.activation(
                            pr[:, kc, :], sps[:], EXP,
                            bias=mask_sb[:, kc:kc + 1], scale=1.0,
                        )
                    else:
                        nc.vector.tensor_scalar(
                            pr[:, kc, :].bitcast(U16), sps[:],
                            SCHR_A, msch_sb[:, kc:kc + 1],
                            op0=MULT, op1=ADD,
                        )
                return pr

            def emit_ctx(hp, qb, hh, pr):
                """ctx^T (+denominator row 64) for head 2*hp+hh, block qb."""
                h = 2 * hp + hh
                ctxt = ctxtp.tile([HD + 1, QB], BF16, name="ctxt")
                for qc in range(QB // QC):
                    cps = wps.tile([128, QC], F32, name="cps", tag="mm")
                    for kc in range(KC):
                        nc.tensor.matmul(
                            cps[:],
                            v_sb[kc][:, h * (HD + 1):h * (HD + 1) + 128],
                            pr[:, kc, qc * QC:(qc + 1) * QC],
                            start=(kc == 0),
                            stop=(kc == KC - 1),
                        )
                    nc.vector.tensor_copy(
                        ctxt[:, qc * QC:(qc + 1) * QC], cps[0:HD + 1, :]
                    )
                return ctxt

            def emit_tail(h, qb, ctxt):
                """transpose ctx^T back to seq-major, normalize into out_sb."""
                E = HD + 2   # 66: keeps each slice 4-byte aligned in PSUM
                tp2 = ctps.tile([128, (QB // 128) * E], BF16, name="tp2")
                for qs in range(QB // 128):
                    nc.tensor.transpose(
                        tp2[:, qs * E:qs * E + HD + 1],
                        ctxt[:, qs * 128:(qs + 1) * 128],
                        identity[0:HD + 1, 0:HD + 1],
                    )
                rd = rdp.tile([128, QB // 128], F32, name="rd")
                nc.vector.reciprocal(rd[:], tp2[:, HD::E])
                for qs in range(QB // 128):
                    sti = qb * (QB // 128) + qs
                    nc.vector.tensor_scalar_mul(
                        out_sb[sti][:, h * HD:(h + 1) * HD],
                        tp2[:, qs * E:qs * E + HD],
                        rd[:, qs:qs + 1],
                    )

            out_r = out_d.ap().rearrange("(t p) n -> p t n", p=128)
            def emit_out_dma(qb):
                half = ST // 2
                nc.sync.dma_start(out_r[:, qb * half:(qb + 1) * half, :], ob[qb][:])

            # ---- main schedule ----
            emit_qk_pair(0)
            pending = []        # deferred (h, qb, ctxt) tails
            fillers = [lambda m=m: emit_qk_pair(m) for m in (1, 2)] + [None] * 4

            for hp in range(NH // 2):
                for qb in range(NQB):
                    prs = [emit_scores(hp, qb, hh) for hh in range(2)]
                    if hp == 0 and qb == 0:
                        for st in range(ST):
                            emit_v_tile(st)
                    else:
                        f = fillers.pop(0)
                        if f is not None:
                            f()
                    for args in pending:
                        emit_tail(*args)
                    if pending and hp == NH // 2 - 1 and qb == NQB - 1:
                        emit_out_dma(0)   # qb0 outputs now complete
                    pending = []
                    for hh in range(2):
                        ctxt = emit_ctx(hp, qb, hh, prs[hh])
                        if hp == NH // 2 - 1 and qb == NQB - 1:
                            emit_tail(2 * hp + hh, qb, ctxt)
                        else:
                            pending.append((2 * hp + hh, qb, ctxt))
            emit_out_dma(1)

    nc.compile()
    return nc


def _get_nc(ck: int):
    if ck not in _nc_cache:
        _nc_cache[ck] = _build(ck)
    return _nc_cache[ck]


def _prepare_in_maps(hidden_states, attention_mask, Wq, bq, Wk, bk, Wv, bv):
    bf16 = mybir.dt.np(BF16)
    hs = np.asarray(hidden_states, dtype=np.float32)
    mask = np.asarray(attention_mask, dtype=np.float32).reshape(B, S)
    wq = np.asarray(Wq, dtype=np.float32) * np.float32(0.125)  # fold 1/sqrt(HD)
    wk = np.asarray(Wk, dtype=np.float32)
    wv = np.asarray(Wv, dtype=np.float32)
    bqs = np.asarray(bq, dtype=np.float32) * np.float32(0.125)
    bks = np.asarray(bk, dtype=np.float32)
    bvs = np.asarray(bv, dtype=np.float32)

    if bqs.any() or bks.any() or bvs.any():
        ck = 7
        pad = ck * 128 - (HID + 1)
        ones = np.ones((B, S, 1), np.float32)
        zer = np.zeros((B, S, pad), np.float32)
        hs = np.concatenate([hs, ones, zer], axis=2)
        def aug(w, b):
            return np.concatenate(
                [w, b[None, :], np.zeros((pad, HID), np.float32)], axis=0)
        wq, wk, wv = aug(wq, bqs), aug(wk, bks), aug(wv, bvs)
    else:
        ck = 6

    wq16 = wq.astype(bf16)
    wk16 = wk.astype(bf16)
    wv16 = wv.astype(bf16)
    msch = (np.float32(SCHR_B) + np.float32(SCHR_A) * mask).astype(np.float32)

    in_maps = []
    for core in range(NCORES):
        b, hg = core // 2, core % 2
        cols = slice(hg * D3, (hg + 1) * D3)
        in_maps.append({
            "hst": np.ascontiguousarray(hs[b].T.astype(bf16)),
            "wq": np.ascontiguousarray(wq16[:, cols]),
            "wk": np.ascontiguousarray(wk16[:, cols]),
            "wv": np.ascontiguousarray(wv16[:, cols]),
            "mask": np.ascontiguousarray(mask[b].reshape(KC, 128).T),
            "msch": np.ascontiguousarray(msch[b].reshape(KC, 128).T),
        })
    return ck, in_maps


def run(hidden_states, attention_mask, Wq, bq, Wk, bk, Wv, bv, **rb_kwargs):
    """Shard, run on 8 cores, gather. Returns (output, BassKernelResults)."""
    ck, in_maps = _prepare_in_maps(
        hidden_states, attention_mask, Wq, bq, Wk, bk, Wv, bv
    )
    nc = _get_nc(ck)
    res = run_bass_kernel_spmd(nc, in_maps, core_ids=list(range(NCORES)), **rb_kwargs)
    out = np.empty((B, S, HID), dtype=np.float32)
    for core in range(NCORES):
        b, hg = core // 2, core % 2
        out[b, :, hg * D3:(hg + 1) * D3] = res.results[core]["out"]
    return out, res


def kernel(hidden_states, attention_mask, Wq, bq, Wk, bk, Wv, bv):
    out, _ = run(hidden_states, attention_mask, Wq, bq, Wk, bk, Wv, bv)
    return out
